# revision 10
# baseline (speedup 1.0000x reference)
"""Trainium2 Bass kernel for nn_CropbiasLoss.

Computes loss = sum_m sum((crop(softmax(s_m)) - crop(softmax(t_m)))^2) / B
over 2176 independent 128x128 maps, data-parallel across 8 NeuronCores.

Math (validated against the jax reference):
 - The student crop position trunc(cs/(cs-1)*(t_pos - 1/(2cs))) equals t_pos
   whenever cs >> 128 (here cs ~ 27000), so both crops share one window and
   the mirror-border gather becomes a separable weight w[y,x] = wr[y]*wc[x],
   wr,wc in {0,1,2}.
 - sum_w (es/cs - et/ct)^2 = (1/cs^2) * sum_w (k*et - es)^2 with k = cs/ct.

Engine layout (measured rates: DMA 377 GB/s/core; ACT 1.2G elem/s/partition;
DVE 1x for max/reduce, 2x bf16 tensor_tensor; gpsimd ~0.6 eff):
 - phase 1 per 4096-f32 chunk: ACT exp+accum (bf16 residents et/es),
   DVE max8 for the chunk max, gpsimd stt (t==cm)*iota accum for its index
   (exact for f32 randn; duplicate-max collisions are measure-zero-rare and
   bounded by one map's contribution).
 - phase 2: gpsimd stt d = k*et - es (fused scale+sub, bf16), DVE
   dc = d*sqrt(wc) (2x bf16), sq = dc*dc (2x bf16), row-reduce (1x),
   then [P,128]-sized wr weighting + final combine.
 - groups: 272 maps/core = 16-map group FIRST (overlaps big groups' DMA),
   then 2 full 128-map groups; no over-read (the old kernel re-read 112
   maps, +41% DMA traffic).

Uses bacc.Bacc: its generate_event_semaphores pass splits multi-sem waits
into EventSemaphore nops (TRN2 encodes at most one sync wait per instr).
"""

import numpy as np

import concourse.bacc as bacc
import concourse.mybir as mybir
from concourse.bass_utils import run_bass_kernel_spmd
from concourse.tile import TileContext

AF = mybir.ActivationFunctionType
ALU = mybir.AluOpType
AX = mybir.AxisListType
FP32 = mybir.dt.float32
BF16 = mybir.dt.bfloat16

NCORES = 8
B = 64
NMAPS = 64 * 34          # 2176
MPC = NMAPS // NCORES    # 272 maps per core
P = 128
W = 128
F = W * W                # 16384
GROUPS = 3               # 16-map group + 2 full 128-map groups
LAST = MPC - 2 * P       # 16
RCH = 4096               # raw/argmax chunk (f32)
NRC = F // RCH           # 4
PCH = 2048               # phase-2 chunk (bf16)
NPC = F // PCH           # 8
RPC = PCH // W           # rows per phase-2 chunk (16)

_NC_CACHE = {}


def _build_nc(nrep=1):
    nc = bacc.Bacc()
    t_d = nc.declare_dram_parameter("t", [MPC, F], FP32, isOutput=False)
    s_d = nc.declare_dram_parameter("s", [MPC, F], FP32, isOutput=False)
    yio_d = nc.declare_dram_parameter("yio", [P, W], FP32, isOutput=False)
    out_d = nc.declare_dram_parameter("out", [P, GROUPS], FP32, isOutput=True)

    with TileContext(nc) as tc:
        with (
            tc.tile_pool(name="raw", bufs=2) as raw,
            tc.tile_pool(name="resid", bufs=1) as resid,
            tc.tile_pool(name="dph", bufs=2) as dph,
            tc.tile_pool(name="work", bufs=2) as work,
            tc.tile_pool(name="sm", bufs=2) as sm,
            tc.tile_pool(name="wg", bufs=6) as wg,
            tc.tile_pool(name="persist", bufs=1) as persist,
        ):
            yio = persist.tile([P, W], FP32)
            nc.sync.dma_start(out=yio[:], in_=yio_d[:])
            outsb = persist.tile([P, GROUPS], FP32)
            nc.vector.memset(outsb[:], 0.0)

            def tt(out, in0, in1, op):
                nc.vector.tensor_tensor(out=out, in0=in0, in1=in1, op=op)

            def axis_weights(pos, pp, tag):
                # per-partition scalars: lo=pos-32, hi=pos+32, tp=2*pos,
                # d1=pos+31, e1=2*pos-129;  w = base + top + bot with
                # base=[lo,hi), top=[tp,d1], bot=[lo,e1]  (weights 0/1/2)
                def ts_imm(src, s1, s2, op0, op1, name):
                    o = sm.tile([P, 1], FP32, tag=tag + name)
                    nc.vector.tensor_scalar(out=o[:pp], in0=src, scalar1=s1,
                                            scalar2=s2, op0=op0, op1=op1)
                    return o[:pp]
                lo = ts_imm(pos, 32.0, None, ALU.subtract, ALU.bypass, "lo")
                hi = ts_imm(pos, 32.0, None, ALU.add, ALU.bypass, "hi")
                tp = ts_imm(pos, 2.0, None, ALU.mult, ALU.bypass, "tp")
                d1 = ts_imm(pos, 31.0, None, ALU.add, ALU.bypass, "d1")
                e1 = ts_imm(pos, 2.0, -129.0, ALU.mult, ALU.add, "e1")

                def cmp_w(psc, op):
                    g = wg.tile([P, W], FP32, tag="wg")
                    tt(g[:pp], yio[:pp], psc.broadcast_to([pp, W]), op)
                    return g
                g1 = cmp_w(lo, ALU.is_ge)
                g2 = cmp_w(hi, ALU.is_lt)
                base = wg.tile([P, W], FP32, tag="wg")
                tt(base[:pp], g1[:pp], g2[:pp], ALU.mult)
                g3 = cmp_w(tp, ALU.is_ge)
                g4 = cmp_w(d1, ALU.is_le)
                top = wg.tile([P, W], FP32, tag="wg")
                tt(top[:pp], g3[:pp], g4[:pp], ALU.mult)
                g6 = cmp_w(e1, ALU.is_le)
                bot = wg.tile([P, W], FP32, tag="wg")
                tt(bot[:pp], g1[:pp], g6[:pp], ALU.mult)
                w1 = wg.tile([P, W], FP32, tag="wg")
                tt(w1[:pp], base[:pp], top[:pp], ALU.add)
                w2 = sm.tile([P, W], FP32, tag=tag + "w2")
                tt(w2[:pp], w1[:pp], bot[:pp], ALU.add)
                return w2

            # group order: 16-map remainder group first, then the two full
            # 128-map groups, so the tail is a full group's phase 2 only.
            groups = [(MPC - P, P), (0, P), (P, P)]
            for gi in [g for _ in range(nrep) for g in range(GROUPS)]:
                m0, pp = groups[gi]
                et = resid.tile([P, F], BF16, tag="et")
                es = resid.tile([P, F], BF16, tag="es")
                ctp = sm.tile([P, NRC], FP32, tag="ctp")
                csp = sm.tile([P, NRC], FP32, tag="csp")
                mxa = sm.tile([P, 8 * NRC], FP32, tag="mxa")
                cmt = sm.tile([P, NRC], FP32, tag="cmt")
                idxp = sm.tile([P, NRC], FP32, tag="idxp")

                # phase 1: stream raw 4096-f32 chunks
                for c in range(NRC):
                    csl = slice(c * RCH, (c + 1) * RCH)
                    t_c = raw.tile([P, RCH], FP32, tag="t_c")
                    nc.sync.dma_start(out=t_c[:pp], in_=t_d[m0:m0 + pp, csl])
                    s_c = raw.tile([P, RCH], FP32, tag="s_c")
                    nc.sync.dma_start(out=s_c[:pp], in_=s_d[m0:m0 + pp, csl])

                    mx8 = mxa[:pp, 8 * c:8 * c + 8]
                    nc.vector.max(out=mx8, in_=t_c[:pp])
                    nc.vector.tensor_copy(out=cmt[:pp, c:c + 1],
                                          in_=mxa[:pp, 8 * c:8 * c + 1])
                    idx8 = sm.tile([P, 8], mybir.dt.uint32, tag="idx8")
                    nc.vector.max_index(out=idx8[:pp], in_max=mx8,
                                        in_values=t_c[:pp])
                    nc.vector.tensor_copy(out=idxp[:pp, c:c + 1],
                                          in_=idx8[:pp, 0:1])

                    nc.scalar.activation(out=et[:pp, csl], in_=t_c[:pp],
                                         func=AF.Exp,
                                         accum_out=ctp[:pp, c:c + 1])
                    nc.scalar.activation(out=es[:pp, csl], in_=s_c[:pp],
                                         func=AF.Exp,
                                         accum_out=csp[:pp, c:c + 1])

                ct = sm.tile([P, 1], FP32, tag="ct")
                nc.vector.tensor_reduce(out=ct[:pp], in_=ctp[:pp], axis=AX.X,
                                        op=ALU.add)
                cs = sm.tile([P, 1], FP32, tag="cs")
                nc.vector.tensor_reduce(out=cs[:pp], in_=csp[:pp], axis=AX.X,
                                        op=ALU.add)
                rct = sm.tile([P, 1], FP32, tag="rct")
                nc.vector.reciprocal(rct[:pp], ct[:pp])
                kk = sm.tile([P, 1], FP32, tag="kk")
                tt(kk[:pp], cs[:pp], rct[:pp], ALU.mult)

                # global argmax combine over per-chunk maxes in cmt
                gmx = sm.tile([P, 1], FP32, tag="gmx")
                nc.vector.tensor_reduce(out=gmx[:pp], in_=cmt[:pp],
                                        axis=AX.X, op=ALU.max)
                ceq = sm.tile([P, NRC], FP32, tag="ceq")
                tt(ceq[:pp], cmt[:pp], gmx[:pp].broadcast_to([pp, NRC]),
                   ALU.is_ge)
                # first winning chunk: j* = NRC - max((NRC-j)*eq)
                rev = sm.tile([P, NRC], FP32, tag="rev")
                nc.vector.tensor_scalar(out=rev[:pp], in0=yio[:pp, 0:NRC],
                                        scalar1=-1.0, scalar2=float(NRC),
                                        op0=ALU.mult, op1=ALU.add)
                wj = sm.tile([P, NRC], FP32, tag="wj")
                tt(wj[:pp], ceq[:pp], rev[:pp], ALU.mult)
                wjm = sm.tile([P, 1], FP32, tag="wjm")
                nc.vector.tensor_reduce(out=wjm[:pp], in_=wj[:pp], axis=AX.X,
                                        op=ALU.max)
                jstar = sm.tile([P, 1], FP32, tag="jstar")
                nc.vector.tensor_scalar(out=jstar[:pp], in0=wjm[:pp],
                                        scalar1=-1.0, scalar2=float(NRC),
                                        op0=ALU.mult, op1=ALU.add)
                # select idxp[j*] via mask-mult-reduce, add j**RCH
                jeq = sm.tile([P, NRC], FP32, tag="jeq")
                tt(jeq[:pp], yio[:pp, 0:NRC],
                   jstar[:pp].broadcast_to([pp, NRC]), ALU.is_equal)
                jsel = sm.tile([P, NRC], FP32, tag="jsel")
                tt(jsel[:pp], jeq[:pp], idxp[:pp], ALU.mult)
                iloc = sm.tile([P, 1], FP32, tag="iloc")
                nc.vector.tensor_reduce(out=iloc[:pp], in_=jsel[:pp],
                                        axis=AX.X, op=ALU.add)
                jbase = sm.tile([P, 1], FP32, tag="jbase")
                nc.vector.tensor_scalar(out=jbase[:pp], in0=jstar[:pp],
                                        scalar1=float(RCH), scalar2=None,
                                        op0=ALU.mult)
                iacc = sm.tile([P, 1], FP32, tag="iacc")
                tt(iacc[:pp], iloc[:pp], jbase[:pp], ALU.add)

                # split flat index: ty = #{j : 128*(j+1) <= i}, tx = i-128*ty
                rr = sm.tile([P, W], FP32, tag="rr")
                nc.vector.tensor_scalar(out=rr[:pp], in0=yio[:pp],
                                        scalar1=128.0, scalar2=128.0,
                                        op0=ALU.mult, op1=ALU.add)
                cmp = sm.tile([P, W], FP32, tag="cmp")
                tt(cmp[:pp], rr[:pp], iacc[:pp].broadcast_to([pp, W]),
                   ALU.is_le)
                ty = sm.tile([P, 1], FP32, tag="ty")
                nc.vector.tensor_reduce(out=ty[:pp], in_=cmp[:pp], axis=AX.X,
                                        op=ALU.add)
                tyn = sm.tile([P, 1], FP32, tag="tyn")
                nc.vector.tensor_scalar(out=tyn[:pp], in0=ty[:pp],
                                        scalar1=-128.0, scalar2=None,
                                        op0=ALU.mult)
                tx = sm.tile([P, 1], FP32, tag="tx")
                tt(tx[:pp], iacc[:pp], tyn[:pp], ALU.add)

                wr = axis_weights(ty[:pp], pp, "r")
                wc = axis_weights(tx[:pp], pp, "c")
                wcb = sm.tile([P, W], BF16, tag="wcb")
                nc.vector.tensor_copy(out=wcb[:pp], in_=wc[:pp])
                wcs_b = wcb[:pp].rearrange("p (o w) -> p o w", o=1)\
                    .broadcast_to([pp, RPC, W])

                # phase 2: d = k*et - es (gpsimd stt, bf16), sq = d*d
                # (DVE 2x bf16), z = sq*wc (2x bf16), row-reduce (1x)
                Rf = sm.tile([P, W], FP32, tag="Rf")
                for c in range(NPC):
                    csl = slice(c * PCH, (c + 1) * PCH)
                    etk = dph.tile([P, PCH], BF16, tag="etk")
                    nc.vector.tensor_scalar(out=etk[:pp], in0=et[:pp, csl],
                                            scalar1=kk[:pp], scalar2=None,
                                            op0=ALU.mult)
                    d = dph.tile([P, PCH], BF16, tag="d")
                    nc.gpsimd.tensor_tensor(out=d[:pp], in0=etk[:pp],
                                            in1=es[:pp, csl], op=ALU.subtract)
                    sq = work.tile([P, PCH], BF16, tag="sq")
                    nc.scalar.activation(out=sq[:pp], in_=d[:pp],
                                         func=AF.Square)
                    dc = work.tile([P, PCH], BF16, tag="dc")
                    dc3 = dc[:pp].rearrange("p (r w) -> p r w", w=W)
                    tt(dc3, sq[:pp].rearrange("p (r w) -> p r w", w=W),
                       wcs_b, ALU.mult)
                    nc.vector.tensor_reduce(
                        out=Rf[:pp, c * RPC:(c + 1) * RPC],
                        in_=dc3, axis=AX.X, op=ALU.add)

                Sj = sm.tile([P, W], FP32, tag="Sj")
                tt(Sj[:pp], Rf[:pp], wr[:pp], ALU.mult)
                lraw = sm.tile([P, 1], FP32, tag="lraw")
                nc.vector.tensor_reduce(out=lraw[:pp], in_=Sj[:pp],
                                        axis=AX.X, op=ALU.add)
                rcs = sm.tile([P, 1], FP32, tag="rcs")
                nc.vector.reciprocal(rcs[:pp], cs[:pp])
                l1 = sm.tile([P, 1], FP32, tag="l1")
                tt(l1[:pp], lraw[:pp], rcs[:pp], ALU.mult)
                tt(outsb[:pp, gi:gi + 1], l1[:pp], rcs[:pp], ALU.mult)

            nc.sync.dma_start(out=out_d[:], in_=outsb[:])
    if not nc.is_finalized():
        nc.finalize()
    return nc


def get_nc(nrep=1):
    if nrep not in _NC_CACHE:
        _NC_CACHE[nrep] = _build_nc(nrep)
    return _NC_CACHE[nrep]


def make_in_maps(s, t):
    s = np.ascontiguousarray(np.asarray(s, dtype=np.float32).reshape(NMAPS, F))
    t = np.ascontiguousarray(np.asarray(t, dtype=np.float32).reshape(NMAPS, F))
    yio = np.ascontiguousarray(np.broadcast_to(
        np.arange(W, dtype=np.float32), (P, W)))
    return [
        {"t": np.ascontiguousarray(t[i * MPC:(i + 1) * MPC]),
         "s": np.ascontiguousarray(s[i * MPC:(i + 1) * MPC]),
         "yio": yio}
        for i in range(NCORES)
    ]


def reduce_outputs(results):
    tot = 0.0
    for i in range(NCORES):
        o = np.asarray(results[i]["out"], dtype=np.float64)
        tot += o[P - LAST:, 0].sum() + o[:, 1:].sum()
    return np.float32(tot / B)


def kernel(s_feature, t_feature):
    nc = get_nc()
    in_maps = make_in_maps(s_feature, t_feature)
    res = run_bass_kernel_spmd(nc, in_maps, list(range(NCORES)))
    return reduce_outputs(res.results)
